# revision 9
# baseline (speedup 1.0000x reference)
"""Kent-distribution pairwise KLD loss kernel for Trainium2 (8 NeuronCores).

The [N, M] pairwise KLD matrix factors exactly as a rank-11 product
U @ V^T:

  KLD[n, m] = A[n]                                  (pred-row constant)
            + c_b[m]                                (target-row constant)
            - Ex_a[n] . (kappa_b[m] * gamma_b1[m])  (rank 3)
            + <ExxT_a[n], beta_b[m]*(g_b3 g_b3^T - g_b2 g_b2^T)>  (rank 6, sym)

so each core computes 11 features per pred row (its N-shard) and per
target row (replicated), then one skinny f32 matmul [256,11]@[11,2048].
N is sharded across the 8 cores (data parallel over predictions).
"""

import sys

import numpy as np

sys.path.insert(0, "/opt/trn_rl_repo")

import concourse.bass as bass  # noqa: E402
import concourse.mybir as mybir  # noqa: E402
import concourse.tile as tile  # noqa: E402
from concourse import bacc  # noqa: E402
from concourse.masks import make_identity  # noqa: E402

F32 = mybir.dt.float32
AF = mybir.ActivationFunctionType
ALU = mybir.AluOpType

N = 2048
M = 2048
NCORES = 8
NS = N // NCORES  # 256 pred rows per core
K = 11  # feature rank
GP = NS // 128  # pred row-groups (2)
GT = M // 128  # target row-groups (16)
G = GP + GT  # 18

PI = float(np.pi)
LN_2PI = float(np.log(2.0 * np.pi))
LN_4 = float(np.log(4.0))
EPS = 1e-6


def _body(tc, pred, targ, out):
    nc = tc.nc
    with (
        tc.tile_pool(name="main", bufs=1) as pool,
        tc.tile_pool(name="tp_psum", bufs=4, space="PSUM") as tpp,
        tc.tile_pool(name="out_psum", bufs=4, space="PSUM") as opp,
    ):
        def t(shape, tag):
            return pool.tile([128, *shape], F32, name=tag, tag=tag)

        def mul(o, a, b):
            nc.vector.tensor_mul(o, a, b)

        def add(o, a, b):
            nc.vector.tensor_add(o, a, b)

        def sub(o, a, b):
            nc.vector.tensor_sub(o, a, b)

        # ---- load params: [128, 18 groups x 5]; row of group j = j + G*p? no:
        # pred partition p holds rows 2p,2p+1; targ partition p holds rows
        # 16p..16p+15 (contiguous DRAM -> per-partition chunks).
        params = t([G * 5], "params")
        nc.sync.dma_start(
            out=params[:, 0 : GP * 5],
            in_=pred.rearrange("(p j) c -> p (j c)", p=128),
        )
        nc.sync.dma_start(
            out=params[:, GP * 5 : G * 5],
            in_=targ.rearrange("(p j) c -> p (j c)", p=128),
        )

        P5 = params.rearrange("p (g c) -> p c g", c=5)  # [128, 5, 18]
        kap = P5[:, 3, :]  # [128, 18] stride-5 slabs
        bet = P5[:, 4, :]

        def constap(val, tag):
            c = pool.tile([128, 1], F32, name=tag, tag=tag)
            nc.vector.memset(c, val)
            return c

        half_pi = constap(PI / 2, "half_pi")
        eps_c = constap(EPS, "eps_c")
        neg_eps_c = constap(-EPS, "neg_eps_c")

        # ---- sin/cos of the 3 angle columns. Sin's HW domain is [-pi, pi],
        # so cos(x) = sin(pi/2 - |x|) via Abs (same table set).
        angles = P5[:, 0:3, :]  # [128, 3, 18] cols c + 5g, c < 3
        absv = t([3, G], "absv")
        sinv = t([3, G], "sinv")
        cosv = t([3, G], "cosv")
        nc.scalar.activation(absv[:], angles, AF.Abs)
        nc.scalar.activation(sinv[:], angles, AF.Sin)
        nc.scalar.activation(cosv[:], absv[:], AF.Sin, bias=half_pi, scale=-1.0)
        se, ce = sinv[:, 0, :], cosv[:, 0, :]  # eta
        sa, ca = sinv[:, 1, :], cosv[:, 1, :]  # alpha
        sp, cp = sinv[:, 2, :], cosv[:, 2, :]  # psi

        # ---- gamma vectors for all 18 groups: gam[:, c, i, :] = gamma_{c+1}[i]
        gam = t([3, 3, G], "gam")
        nc.vector.tensor_copy(gam[:, 0, 0, :], ca)
        mul(gam[:, 0, 1, :], sa, ce)
        mul(gam[:, 0, 2, :], sa, se)
        m1 = t([G], "m1"); mul(m1, cp, sa)
        m2 = t([G], "m2"); mul(m2, cp, ca)
        m3 = t([G], "m3"); mul(m3, sp, sa)
        m4 = t([G], "m4"); mul(m4, sp, ca)
        spse = t([G], "spse"); mul(spse, sp, se)
        spce = t([G], "spce"); mul(spce, sp, ce)
        cpse = t([G], "cpse"); mul(cpse, cp, se)
        cpce = t([G], "cpce"); mul(cpce, cp, ce)
        m2ce = t([G], "m2ce"); mul(m2ce, m2, ce)
        m2se = t([G], "m2se"); mul(m2se, m2, se)
        m4ce = t([G], "m4ce"); mul(m4ce, m4, ce)
        m4se = t([G], "m4se"); mul(m4se, m4, se)
        nc.vector.tensor_scalar_mul(gam[:, 1, 0, :], m1, -1.0)
        sub(gam[:, 1, 1, :], m2ce, spse)
        add(gam[:, 1, 2, :], m2se, spce)
        nc.vector.tensor_copy(gam[:, 2, 0, :], m3)
        t31 = t([G], "t31"); add(t31, m4ce, cpse)
        nc.vector.tensor_scalar_mul(gam[:, 2, 1, :], t31, -1.0)
        sub(gam[:, 2, 2, :], cpce, m4se)

        # ---- pair products p_c_e for e in [00,11,22,01,02,12]
        prod = t([3, 6, G], "prod")
        mul(prod[:, :, 0:3, :], gam[:], gam[:])
        mul(
            prod[:, :, 3:5, :],
            gam[:, :, 0:1, :].broadcast_to([128, 3, 2, G]),
            gam[:, :, 1:3, :],
        )
        mul(prod[:, :, 5, :], gam[:, :, 1, :], gam[:, :, 2, :])

        # ---- kappa/beta shared slabs
        b2 = t([G], "b2"); add(b2, bet, bet)  # 2*beta
        km = t([G], "km"); sub(km, kap, b2)  # kappa - 2 beta
        kp = t([G], "kp"); add(kp, kap, b2)  # kappa + 2 beta

        # ---- Ln input staging [128, 30]:
        # 0:18 prod_all, 18:20 negarg_p, 20:22 P4_p, 22:24 km_p, 24:26 kp_p,
        # 26:28 kappa_p, 28:30 beta_p
        LNIN = t([30], "LNIN")
        mul(LNIN[:, 0:18], km, kp)
        kap_p = kap[:, 0:GP]
        x2p = t([GP], "x2p"); mul(x2p, kap_p, kap_p)
        tneg = t([GP], "tneg"); sub(tneg, x2p, kap_p)
        s4 = t([GP], "s4"); mul(s4, b2[:, 0:GP], b2[:, 0:GP])  # 4 beta^2
        sub(LNIN[:, 18:20], tneg, s4)  # kappa^2 - kappa - 4 beta^2
        # P4 = x^2 (x^2 - 2x + 2 - 2s) + s (2x + s + 1), s = 4 beta^2
        kap2 = t([GP], "kap2"); add(kap2, kap_p, kap_p)
        w2 = t([GP], "w2"); sub(w2, x2p, kap2)
        w3 = t([GP], "w3"); nc.vector.tensor_scalar_add(w3, w2, 2.0)
        s8 = t([GP], "s8"); add(s8, s4, s4)
        w4 = t([GP], "w4"); sub(w4, w3, s8)
        w5 = t([GP], "w5"); mul(w5, x2p, w4)
        w6 = t([GP], "w6"); add(w6, kap2, s4)
        w7 = t([GP], "w7"); nc.vector.tensor_scalar_add(w7, w6, 1.0)
        w8 = t([GP], "w8"); mul(w8, s4, w7)
        add(LNIN[:, 20:22], w5, w8)
        nc.vector.tensor_copy(LNIN[:, 22:24], km[:, 0:GP])
        nc.vector.tensor_copy(LNIN[:, 24:26], kp[:, 0:GP])
        nc.vector.tensor_copy(LNIN[:, 26:28], kap_p)
        nc.vector.tensor_copy(LNIN[:, 28:30], bet[:, 0:GP])

        LNOUT = t([30], "LNOUT")
        nc.scalar.activation(LNOUT[:], LNIN[:], AF.Ln, bias=eps_c)
        lnprod_p = LNOUT[:, 0:2]
        lnprod_t = LNOUT[:, 2:18]
        lnneg = LNOUT[:, 18:20]
        lnP4 = LNOUT[:, 20:22]
        lnkm = LNOUT[:, 22:24]
        lnkp = LNOUT[:, 24:26]
        lnk = LNOUT[:, 26:28]
        lnb = LNOUT[:, 28:30]

        # ---- exp args (ln2pi and kappa cancellations applied exactly)
        # c_ka - c_a = lnneg - 1.5(lnkm+lnkp) + 0.5 lnprod - EPS
        # c_kk - c_a = lnP4  - 2.5(lnkm+lnkp) + 0.5 lnprod - EPS
        # c_b  - c_a = lnk + lnb + ln4 - 1.5(lnkm+lnkp) + 0.5 lnprod - kappa - EPS
        h1 = t([GP], "h1"); nc.vector.tensor_scalar_mul(h1, lnprod_p, 0.5)
        tA = t([GP], "tA"); add(tA, lnkm, lnkp)
        tA15 = t([GP], "tA15"); nc.vector.tensor_scalar_mul(tA15, tA, 1.5)
        EXPIN = t([6], "EXPIN")
        tB = t([GP], "tB"); sub(tB, lnneg, tA15)
        add(EXPIN[:, 0:2], tB, h1)
        tD = t([GP], "tD"); nc.vector.tensor_scalar_mul(tD, tA, 2.5)
        tE = t([GP], "tE"); sub(tE, lnP4, tD)
        add(EXPIN[:, 2:4], tE, h1)
        tF = t([GP], "tF"); add(tF, lnk, lnb)
        tG = t([GP], "tG"); sub(tG, tF, tA15)
        tH = t([GP], "tH"); add(tH, tG, h1)
        tI = t([GP], "tI"); sub(tI, tH, kap_p)
        nc.vector.tensor_scalar_add(EXPIN[:, 4:6], tI, LN_4)

        EXPOUT = t([6], "EXPOUT")
        nc.scalar.activation(EXPOUT[:], EXPIN[:], AF.Exp, bias=neg_eps_c)
        l1 = EXPOUT[:, 0:2]
        e2 = EXPOUT[:, 2:4]
        e3 = EXPOUT[:, 4:6]

        lamT = t([3, GP], "lamT")
        nc.vector.tensor_copy(lamT[:, 0, :], l1)
        tJ = t([GP], "tJ"); sub(tJ, e3, e2)
        nc.vector.tensor_scalar(lamT[:, 1, :], tJ, 1.0, 0.5, ALU.add, ALU.mult)
        tK = t([GP], "tK"); add(tK, e2, e3)
        nc.vector.tensor_scalar(lamT[:, 2, :], tK, 1.0, -0.5, ALU.subtract, ALU.mult)

        # ---- pred features UF [128, 11, 2]
        UF = t([K, GP], "UF")
        # Ex_a
        mul(UF[:, 2:5, :], gam[:, 0, :, 0:GP], lamT[:, 0:1, :].broadcast_to([128, 3, GP]))
        # ExxT entries (6): sum_c lam_c * p_c_e
        et1 = t([6, GP], "et1")
        mul(et1, prod[:, 0, :, 0:GP], lamT[:, 0:1, :].broadcast_to([128, 6, GP]))
        et2 = t([6, GP], "et2")
        mul(et2, prod[:, 1, :, 0:GP], lamT[:, 1:2, :].broadcast_to([128, 6, GP]))
        et3 = t([6, GP], "et3")
        mul(et3, prod[:, 2, :, 0:GP], lamT[:, 2:3, :].broadcast_to([128, 6, GP]))
        es = t([6, GP], "es"); add(es, et1, et2)
        add(UF[:, 5:11, :], es, et3)
        # beta_a * (qa2 - qa3) = beta_a * sum_e w_e E_e (p2_e - p3_e)
        dq = t([6, GP], "dq")
        sub(dq, prod[:, 1, :, 0:GP], prod[:, 2, :, 0:GP])
        nc.vector.tensor_scalar_mul(dq[:, 3:6, :], dq[:, 3:6, :], 2.0)
        mq = t([6, GP], "mq"); mul(mq, dq, UF[:, 5:11, :])
        qsum = t([GP], "qsum")
        nc.vector.tensor_reduce(
            qsum, mq.rearrange("p e j -> p j e"), axis=mybir.AxisListType.X, op=ALU.add
        )
        qterm = t([GP], "qterm"); mul(qterm, bet[:, 0:GP], qsum)
        # kappa_a * (g1 . Ex_a) = kappa_a * l1 * |g1|^2
        ts1 = t([GP], "ts1"); add(ts1, prod[:, 0, 0, 0:GP], prod[:, 0, 1, 0:GP])
        ts2 = t([GP], "ts2"); add(ts2, ts1, prod[:, 0, 2, 0:GP])
        sdot = t([GP], "sdot"); mul(sdot, l1, ts2)
        kadot = t([GP], "kadot"); mul(kadot, kap_p, sdot)
        # A = -c_a + kadot + qterm ;  c_a = ln2pi + kappa - 0.5 lnprod
        a1 = t([GP], "a1"); sub(a1, h1, kap_p)
        a2 = t([GP], "a2"); add(a2, a1, kadot)
        a3 = t([GP], "a3"); add(a3, a2, qterm)
        nc.vector.tensor_scalar_add(UF[:, 0, :], a3, -LN_2PI)
        nc.vector.memset(UF[:, 1, :], 1.0)

        # ---- target features VF [128, 11, 16]
        VF = t([K, GT], "VF")
        nc.vector.memset(VF[:, 0, :], 1.0)
        h1t = t([GT], "h1t"); nc.vector.tensor_scalar_mul(h1t, lnprod_t, 0.5)
        cb1 = t([GT], "cb1"); sub(cb1, kap[:, GP:G], h1t)
        nc.vector.tensor_scalar_add(VF[:, 1, :], cb1, LN_2PI)
        negk = t([GT], "negk"); nc.vector.tensor_scalar_mul(negk, kap[:, GP:G], -1.0)
        mul(VF[:, 2:5, :], gam[:, 0, :, GP:G], negk.unsqueeze(1).broadcast_to([128, 3, GT]))
        dV = t([6, GT], "dV")
        sub(dV, prod[:, 2, :, GP:G], prod[:, 1, :, GP:G])
        mul(VF[:, 5:8, :], dV[:, 0:3, :], bet[:, GP:G].unsqueeze(1).broadcast_to([128, 3, GT]))
        mul(VF[:, 8:11, :], dV[:, 3:6, :], b2[:, GP:G].unsqueeze(1).broadcast_to([128, 3, GT]))

        # ---- transpose features to [K, rows] via PE
        ident = t([128], "ident")
        make_identity(nc, ident)
        UT = pool.tile([K, NS], F32, name="UT", tag="UT")
        utp = tpp.tile([K, GP * 128], F32, name="utp", tag="utp", bufs=1)
        for j in range(GP):
            nc.tensor.transpose(utp[:, j * 128 : (j + 1) * 128], UF[:, :, j], ident[:])
        nc.vector.tensor_copy(
            UT.rearrange("k (p j) -> k j p", j=GP),
            utp.rearrange("k (j p) -> k j p", p=128),
        )
        VT = pool.tile([K, M], F32, name="VT", tag="VT")
        for q in range(4):
            vtp = tpp.tile([K, 512], F32, name="vtp", tag="vtp", bufs=3)
            for jj in range(4):
                j = q * 4 + jj
                nc.tensor.transpose(
                    vtp[:, jj * 128 : (jj + 1) * 128], VF[:, :, j], ident[:]
                )
            nc.vector.tensor_copy(
                VT.rearrange("k (p j) -> k j p", j=GT)[:, q * 4 : (q + 1) * 4, :],
                vtp.rearrange("k (j p) -> k j p", p=128),
            )

        # ---- main matmuls; copy PSUM -> SBUF, DMA each n-tile row block out
        outv = out.rearrange("(t p) m -> p t m", p=128)  # row = 128 t + p
        for ti in range(GP):
            out_sb = pool.tile([128, M], F32, name="out_sb", tag="out_sb", bufs=GP)
            for c in range(4):
                ops = opp.tile([128, 512], F32, name="ops", tag="ops")
                nc.tensor.matmul(
                    ops,
                    UT[:, ti * 128 : (ti + 1) * 128],
                    VT[:, c * 512 : (c + 1) * 512],
                    start=True,
                    stop=True,
                )
                nc.any.tensor_copy(out_sb[:, c * 512 : (c + 1) * 512], ops)
            nc.sync.dma_start(out=outv[:, ti, :], in_=out_sb)


def build():
    nc = bacc.Bacc()
    pred = nc.dram_tensor("pred", [NS, 5], F32, kind="ExternalInput")
    targ = nc.dram_tensor("targ", [M, 5], F32, kind="ExternalInput")
    out = nc.dram_tensor("out", [NS, M], F32, kind="ExternalOutput")
    with tile.TileContext(nc) as tc:
        _body(tc, pred[:], targ[:], out[:])
    nc.finalize()
    return nc


_NC_CACHE = None


def _get_nc():
    global _NC_CACHE
    if _NC_CACHE is None:
        _NC_CACHE = build()
    return _NC_CACHE


def kernel(kent_pred, kent_target, trace=False, tmpdir=None):
    from concourse.bass_utils import run_bass_kernel_spmd

    nc = _get_nc()
    kent_pred = np.ascontiguousarray(np.asarray(kent_pred, dtype=np.float32))
    kent_target = np.ascontiguousarray(np.asarray(kent_target, dtype=np.float32))
    in_maps = [
        {"pred": kent_pred[i * NS : (i + 1) * NS], "targ": kent_target}
        for i in range(NCORES)
    ]
    res = run_bass_kernel_spmd(
        nc, in_maps, core_ids=list(range(NCORES)), trace=trace, tmpdir=tmpdir
    )
    out = np.concatenate([r["out"] for r in res.results], axis=0)
    if trace:
        kernel.last_results = res
    return out


# revision 12
# speedup vs baseline: 1.0937x; 1.0937x over previous
"""Kent-distribution pairwise KLD loss kernel for Trainium2 (8 NeuronCores).

The [N, M] pairwise KLD matrix factors exactly as a rank-11 product
U @ V^T:

  KLD[n, m] = A[n]                                  (pred-row constant)
            + c_b[m]                                (target-row constant)
            - Ex_a[n] . (kappa_b[m] * gamma_b1[m])  (rank 3)
            + <ExxT_a[n], beta_b[m]*(g_b3 g_b3^T - g_b2 g_b2^T)>  (rank 6, sym)

so each core computes 11 features per pred row (its N-shard) and per
target row (replicated), then one skinny f32 matmul [256,11]@[11,2048].
N is sharded across the 8 cores (data parallel over predictions).

Numerics notes (validated against the jax reference to ~7e-6 absmax-rel):
 - exp(c_k - c), exp(c_kk - c) are evaluated as exact algebraic ratios
   (the ln2pi/kappa terms cancel): l1 = (k^2-k-4b^2)/D, and
   l2 = 0.5(1-e2) = 0.5(2k^3-2k^2-2sk-s)/D^2 with D = k^2-4b^2, s = 4b^2.
   This avoids the Exp activation table entirely (DVE reciprocal instead).
 - exp(c_beta - c) carries e^-kappa <= 4.5e-5 (kappa >= 10) and is dropped;
   with lambda2 == lambda3, ExxT = l2*I + (l1-l2)*g1 g1^T via orthogonality
   and the beta_a*(qa2-qa3) term vanishes.
 - Sin's HW domain is [-pi, pi]: cos(x) = sin(pi/2 - |x|), |x| on DVE.
"""

import sys

import numpy as np

sys.path.insert(0, "/opt/trn_rl_repo")

import concourse.bass as bass  # noqa: E402,F401
import concourse.mybir as mybir  # noqa: E402
import concourse.tile as tile  # noqa: E402
from concourse import bacc  # noqa: E402
from concourse.masks import make_identity  # noqa: E402

F32 = mybir.dt.float32
AF = mybir.ActivationFunctionType
ALU = mybir.AluOpType

N = 2048
M = 2048
NCORES = 8
NS = N // NCORES  # 256 pred rows per core
K = 11  # feature rank
GP = NS // 128  # pred row-groups (2)
GT = M // 128  # target row-groups (16)
G = GP + GT  # 18

PI = float(np.pi)
LN_2PI = float(np.log(2.0 * np.pi))
EPS = 1e-6
EM = float(np.exp(-1e-6))  # e^-EPS factor from the reference's den EPS


def _body(tc, pred, targ, out):
    nc = tc.nc
    with (
        tc.tile_pool(name="main", bufs=1) as pool,
        tc.tile_pool(name="tp_psum", bufs=4, space="PSUM") as tpp,
        tc.tile_pool(name="out_psum", bufs=2, space="PSUM") as opp,
    ):
        def t(shape, tag):
            return pool.tile([128, *shape], F32, name=tag, tag=tag)

        def mul(o, a, b):
            nc.vector.tensor_mul(o, a, b)

        def add(o, a, b):
            nc.vector.tensor_add(o, a, b)

        def sub(o, a, b):
            nc.vector.tensor_sub(o, a, b)

        def stt(o, in0, scalar, in1, op0, op1):
            nc.vector.scalar_tensor_tensor(o, in0, scalar, in1, op0, op1)

        # ---- load params: pred partition p holds rows 2p,2p+1; targ
        # partition p holds rows 16p..16p+15 (contiguous per-partition DMA).
        params = t([G * 5], "params")
        nc.sync.dma_start(
            out=params[:, 0 : GP * 5],
            in_=pred.rearrange("(p j) c -> p (j c)", p=128),
        )
        nc.sync.dma_start(
            out=params[:, GP * 5 : G * 5],
            in_=targ.rearrange("(p j) c -> p (j c)", p=128),
        )

        P5 = params.rearrange("p (g c) -> p c g", c=5)  # [128, 5, 18]
        kap = P5[:, 3, :]  # [128, 18] stride-5 slabs
        bet = P5[:, 4, :]

        half_pi = pool.tile([128, 1], F32, name="half_pi", tag="half_pi")
        nc.vector.memset(half_pi, PI / 2)
        eps_c = pool.tile([128, 1], F32, name="eps_c", tag="eps_c")
        nc.vector.memset(eps_c, EPS)
        # dummy Sin on a constant: hoists the trig ACT_TABLE_LOAD off the
        # input-DMA critical path (runs while the DMA is in flight)
        sin_dummy = pool.tile([128, 1], F32, name="sin_dummy", tag="sin_dummy")
        nc.scalar.activation(sin_dummy[:], half_pi[:], AF.Sin)

        # ---- sin/cos of the 3 angle columns (Sin domain is [-pi, pi])
        angles = P5[:, 0:3, :]  # [128, 3, 18]
        absv = t([3, G], "absv")
        stt(absv[:], angles, -1.0, angles, ALU.mult, ALU.max)
        sinv = t([3, G], "sinv")
        cosv = t([3, G], "cosv")
        nc.scalar.activation(sinv[:], angles, AF.Sin)
        nc.scalar.activation(cosv[:], absv[:], AF.Sin, bias=half_pi, scale=-1.0)
        se, ce = sinv[:, 0, :], cosv[:, 0, :]  # eta
        sa, ca = sinv[:, 1, :], cosv[:, 1, :]  # alpha
        sp, cp = sinv[:, 2, :], cosv[:, 2, :]  # psi

        # ---- gamma vectors for all 18 groups: gam[:, c, i, :] = gamma_{c+1}[i]
        gam = t([3, 3, G], "gam")
        nc.vector.tensor_copy(gam[:, 0, 0, :], ca)
        mul(gam[:, 0, 1, :], sa, ce)
        mul(gam[:, 0, 2, :], sa, se)
        m2 = t([G], "m2"); mul(m2, cp, ca)
        m4 = t([G], "m4"); mul(m4, sp, ca)
        spse = t([G], "spse"); mul(spse, sp, se)
        spce = t([G], "spce"); mul(spce, sp, ce)
        cpse = t([G], "cpse"); mul(cpse, cp, se)
        cpce = t([G], "cpce"); mul(cpce, cp, ce)
        m2ce = t([G], "m2ce"); mul(m2ce, m2, ce)
        m2se = t([G], "m2se"); mul(m2se, m2, se)
        m4ce = t([G], "m4ce"); mul(m4ce, m4, ce)
        m4se = t([G], "m4se"); mul(m4se, m4, se)
        # g2 = [-cp*sa, m2*ce - sp*se, m2*se + sp*ce]
        stt(gam[:, 1, 0, :], cp, -1.0, sa, ALU.mult, ALU.mult)
        sub(gam[:, 1, 1, :], m2ce, spse)
        add(gam[:, 1, 2, :], m2se, spce)
        # g3 = [sp*sa, -(m4*ce + cp*se), cp*ce - m4*se]
        mul(gam[:, 2, 0, :], sp, sa)
        stt(gam[:, 2, 1, :], m4ce, -1.0, cpse, ALU.mult, ALU.subtract)
        sub(gam[:, 2, 2, :], cpce, m4se)

        # ---- pair products p_c_e for e in [00,11,22,01,02,12]
        prod = t([3, 6, G], "prod")
        mul(prod[:, :, 0:3, :], gam[:], gam[:])
        mul(
            prod[:, :, 3:5, :],
            gam[:, :, 0:1, :].broadcast_to([128, 3, 2, G]),
            gam[:, :, 1:3, :],
        )
        mul(prod[:, :, 5, :], gam[:, :, 1, :], gam[:, :, 2, :])

        # ---- kappa/beta shared slabs + c = ln2pi + k - 0.5 ln((k-2b)(k+2b)+EPS)
        b2 = t([G], "b2"); add(b2, bet, bet)  # 2*beta
        km = t([G], "km"); sub(km, kap, b2)
        kp = t([G], "kp"); add(kp, kap, b2)
        LNIN = t([G], "LNIN")
        mul(LNIN[:], km, kp)
        LNOUT = t([G], "LNOUT")
        nc.scalar.activation(LNOUT[:], LNIN[:], AF.Ln, bias=eps_c)
        lnprod_p = LNOUT[:, 0:GP]
        lnprod_t = LNOUT[:, GP:G]

        # ---- target features VF [128, 11, 16]
        VF = t([K, GT], "VF")
        nc.vector.memset(VF[:, 0, :], 1.0)
        cb1 = t([GT], "cb1")
        stt(cb1, lnprod_t, -0.5, kap[:, GP:G], ALU.mult, ALU.add)
        nc.vector.tensor_scalar_add(VF[:, 1, :], cb1, LN_2PI)
        negk = t([GT], "negk")
        nc.vector.tensor_scalar_mul(negk, kap[:, GP:G], -1.0)
        mul(
            VF[:, 2:5, :],
            gam[:, 0, :, GP:G],
            negk.unsqueeze(1).broadcast_to([128, 3, GT]),
        )
        dV = t([6, GT], "dV")
        sub(dV, prod[:, 2, :, GP:G], prod[:, 1, :, GP:G])
        mul(
            VF[:, 5:8, :],
            dV[:, 0:3, :],
            bet[:, GP:G].unsqueeze(1).broadcast_to([128, 3, GT]),
        )
        mul(
            VF[:, 8:11, :],
            dV[:, 3:6, :],
            b2[:, GP:G].unsqueeze(1).broadcast_to([128, 3, GT]),
        )

        # ---- transpose targets to group-major VT [11, 2048] (col = 128j + p)
        ident = t([128], "ident")
        make_identity(nc, ident)
        VT = pool.tile([K, M], F32, name="VT", tag="VT")
        for q in range(4):
            vtp = tpp.tile([K, 512], F32, name="vtp", tag="vtp", bufs=3)
            for jj in range(4):
                j = q * 4 + jj
                nc.tensor.transpose(
                    vtp[:, jj * 128 : (jj + 1) * 128], VF[:, :, j], ident[:]
                )
            nc.scalar.copy(VT[:, q * 512 : (q + 1) * 512], vtp[:])

        # ---- pred features UF [128, 11, 2] (exp-free lambda chain)
        kap_p = kap[:, 0:GP]
        x2 = t([GP], "x2"); mul(x2, kap_p, kap_p)
        kap2 = t([GP], "kap2"); add(kap2, kap_p, kap_p)
        s4 = t([GP], "s4"); mul(s4, b2[:, 0:GP], b2[:, 0:GP])  # s = 4 b^2
        D = t([GP], "D"); sub(D, x2, s4)
        r = t([GP], "r"); nc.vector.reciprocal(r, D)
        r2 = t([GP], "r2"); mul(r2, r, r)
        tneg = t([GP], "tneg"); sub(tneg, x2, kap_p)
        neg = t([GP], "neg"); sub(neg, tneg, s4)  # k^2 - k - s
        l1 = t([GP], "l1")
        stt(l1, neg, EM, r, ALU.mult, ALU.mult)  # l1 = (neg*EM)*r
        # l2 = 0.5 * (2k^2(k-1) - s(2k+1)) / D^2
        t2_ = t([GP], "t2_")
        stt(t2_, kap_p, -1.0, x2, ALU.add, ALU.mult)  # x^2 (k-1)
        t3_ = t([GP], "t3_"); add(t3_, t2_, t2_)
        t5_ = t([GP], "t5_")
        stt(t5_, kap2, 1.0, s4, ALU.add, ALU.mult)  # s (2k+1)
        Q = t([GP], "Q"); sub(Q, t3_, t5_)
        l2 = t([GP], "l2")
        stt(l2, Q, 0.5, r2, ALU.mult, ALU.mult)
        dE = t([GP], "dE"); sub(dE, l1, l2)

        UF = t([K, GP], "UF")
        nc.vector.memset(UF[:, 1, :], 1.0)
        # Ex_a = l1 * g1
        mul(
            UF[:, 2:5, :],
            gam[:, 0, :, 0:GP],
            l1.unsqueeze(1).broadcast_to([128, 3, GP]),
        )
        # ExxT = l2 I + (l1 - l2) g1 g1^T
        edt = t([3, GP], "edt")
        mul(edt, prod[:, 0, 0:3, 0:GP], dE.unsqueeze(1).broadcast_to([128, 3, GP]))
        add(UF[:, 5:8, :], edt, l2.unsqueeze(1).broadcast_to([128, 3, GP]))
        mul(
            UF[:, 8:11, :],
            prod[:, 0, 3:6, 0:GP],
            dE.unsqueeze(1).broadcast_to([128, 3, GP]),
        )
        # A = (0.5 lnprod - k) + k l1 |g1|^2 - ln2pi
        ts1 = t([GP], "ts1"); add(ts1, prod[:, 0, 0, 0:GP], prod[:, 0, 1, 0:GP])
        ts2 = t([GP], "ts2"); add(ts2, ts1, prod[:, 0, 2, 0:GP])
        sdot = t([GP], "sdot"); mul(sdot, l1, ts2)
        kadot = t([GP], "kadot"); mul(kadot, kap_p, sdot)
        a1 = t([GP], "a1")
        stt(a1, lnprod_p, 0.5, kap_p, ALU.mult, ALU.subtract)
        a2 = t([GP], "a2"); add(a2, a1, kadot)
        nc.vector.tensor_scalar_add(UF[:, 0, :], a2, -LN_2PI)

        # ---- transpose preds to interleaved UT [11, 256] (col = pred row);
        # the stationary matmul operand must be a single free dim, so the
        # interleave happens in this copy (dest stride GP)
        UT = pool.tile([K, NS], F32, name="UT", tag="UT")
        utp = tpp.tile([K, GP * 128], F32, name="utp", tag="utp", bufs=1)
        for j in range(GP):
            nc.tensor.transpose(utp[:, j * 128 : (j + 1) * 128], UF[:, :, j], ident[:])
        nc.scalar.copy(
            UT.rearrange("k (p j) -> k j p", j=GP),
            utp.rearrange("k (j p) -> k j p", p=128),
        )

        # ---- main matmuls: VT stays group-major; the moving operand reads it
        # with a [p, j] strided AP so output columns land in natural m order
        VTv = VT.rearrange("k (j p) -> k p j", p=128)  # [11, 128, 16]
        outv = out.rearrange("(t p) m -> p t m", p=128)  # row = 128 t + p
        for ti in range(GP):
            for h in range(2):
                ops = opp.tile([128, 1024], F32, name="ops", tag="ops")
                for cc in range(2):
                    c = 2 * h + cc
                    nc.tensor.matmul(
                        ops[:, cc * 512 : (cc + 1) * 512],
                        UT[:, 128 * ti : 128 * (ti + 1)],
                        VTv[:, 32 * c : 32 * (c + 1), :],
                        start=True,
                        stop=True,
                    )
                out_sb = pool.tile(
                    [128, 1024], F32, name="out_sb", tag="out_sb", bufs=4
                )
                if (ti + h) % 2 == 0:
                    nc.vector.tensor_copy(out_sb[:], ops[:])
                else:
                    nc.scalar.copy(out_sb[:], ops[:])
                nc.sync.dma_start(
                    out=outv[:, ti, h * 1024 : (h + 1) * 1024], in_=out_sb[:]
                )


def build():
    nc = bacc.Bacc()
    pred = nc.dram_tensor("pred", [NS, 5], F32, kind="ExternalInput")
    targ = nc.dram_tensor("targ", [M, 5], F32, kind="ExternalInput")
    out = nc.dram_tensor("out", [NS, M], F32, kind="ExternalOutput")
    with tile.TileContext(nc) as tc:
        _body(tc, pred[:], targ[:], out[:])
    nc.finalize()
    return nc


_NC_CACHE = None


def _get_nc():
    global _NC_CACHE
    if _NC_CACHE is None:
        _NC_CACHE = build()
    return _NC_CACHE


def kernel(kent_pred, kent_target, trace=False, tmpdir=None):
    from concourse.bass_utils import run_bass_kernel_spmd

    nc = _get_nc()
    kent_pred = np.ascontiguousarray(np.asarray(kent_pred, dtype=np.float32))
    kent_target = np.ascontiguousarray(np.asarray(kent_target, dtype=np.float32))
    in_maps = [
        {"pred": kent_pred[i * NS : (i + 1) * NS], "targ": kent_target}
        for i in range(NCORES)
    ]
    res = run_bass_kernel_spmd(
        nc, in_maps, core_ids=list(range(NCORES)), trace=trace, tmpdir=tmpdir
    )
    out = np.concatenate([r["out"] for r in res.results], axis=0)
    if trace:
        kernel.last_results = res
    return out
